# revision 1
# baseline (speedup 1.0000x reference)
"""GAT (2-layer, PyG-style) Trainium2 Bass kernel, 8-core SPMD.

Strategy (edge parallelism by destination):
  - Add self loops, sort edges by dst, partition dst-node blocks of 128
    across 8 cores (contiguous block ranges).
  - Per layer, a node-feature table T (row-per-node: [h | a_src | a_dst],
    bf16 h + f32 attn scalars, 256B-multiple row stride) is built on
    device (distributed across cores), assembled on host, and re-fed
    replicated to every core.
  - Edge phase per core: for each of its dst blocks, batched dma_gather
    of T[src] rows (int16 idx limit 32767 -> two gathers split by
    src < 32768), then per 128-edge tile:
      S[e,d] = (dstloc_e == d)                 (DVE tensor_scalar is_equal)
      ST = S^T                                 (PE transpose)
      u = ST.T @ a_dst_block + a_src_gathered  (PE matmul + DVE add)
      p = exp(leaky_relu(u))                   (ACT Lrelu, ACT Exp)
      M = h_gathered * p (per-head broadcast)  (DVE)
      acc[d, :] += S.T @ [M | p]               (PE matmul, PSUM accumulate)
    Segment softmax without max-subtraction (logits are O(10), exact in
    f32: softmax is shift-invariant so this matches the reference).
  - Block epilogue: out = acc[:, :HC] / acc[:, HC:] (per head), + bias,
    ELU (layer 1) or head-mean (layer 2).

Three launches (host assembles/replicates tables between them; that
host work is pure data movement, all math is on device):
  A0: x_slice @ W1 -> per-node h1/as1/ad1 slice            (distributed)
  A1: layer-1 edge phase -> h1' slice; h1' @ W2 -> h2/as2/ad2 slice
  B1: layer-2 edge phase -> final out slice
"""

import sys

sys.path.insert(0, "/opt/trn_rl_repo")

import math
import numpy as np
import ml_dtypes

import concourse.bass as bass
import concourse.bacc as bacc
import concourse.tile as tile
from concourse import mybir
from concourse.bass_utils import run_bass_kernel_spmd
from concourse.masks import make_identity

BF16 = ml_dtypes.bfloat16
F32 = mybir.dt.float32
BF = mybir.dt.bfloat16
I16 = mybir.dt.int16
I32 = mybir.dt.int32

P = 128
NCORES = 8
SPLIT = 32768
NEG_SLOPE = 0.2
PAD_DST = 1000.0  # dstloc sentinel: matches no d in [0,128)
IOTA = np.tile(np.arange(P, dtype=np.float32), (P, 1)).astype(ml_dtypes.bfloat16)


def _cfg(N, E, IN, H1, C1, H2, C2):
    nblk = math.ceil(N / P)
    slots = math.ceil(nblk / NCORES)
    return dict(
        N=N, E=E, IN=IN, H1=H1, C1=C1, H2=H2, C2=C2,
        D1=H1 * C1, D2=H2 * C2,
        NBLK=nblk, SLOTS=slots, NPC=slots * P, NPAD=nblk * P,
        # table row lengths in bf16 elems (256B-multiple strides)
        ROW1=_row_elems(H1 * C1 + 2 * 2 * H1),  # h bf16 + as,ad f32
        ROW2=_row_elems(H2 * C2 + 2 * 2 * H2),
    )


def _row_elems(used_bf16_elems):
    # round row up to a multiple of 128 bf16 elems (256 bytes)
    return ((used_bf16_elems + 127) // 128) * 128


CFG = _cfg(N=50000, E=800000, IN=128, H1=4, C1=32, H2=8, C2=32)


# ---------------------------------------------------------------------------
# Host-side edge plan
# ---------------------------------------------------------------------------

def build_edge_plan(cfg, src, dst):
    """Sort by dst, bucket into (core, slot) dst blocks, split each block's
    edges by src < SPLIT, pad each group to a multiple of 128.

    Returns a static `plan` (identical across cores: per-slot tile counts
    and call descriptors) plus per-core data buffers (gather indices,
    local-dst per tile, valid counts)."""
    slots, nblk = cfg["SLOTS"], cfg["NBLK"]
    order = np.argsort(dst, kind="stable")
    ss = src[order].astype(np.int64)
    dd = dst[order].astype(np.int64)
    blk_edges = {}
    bounds = np.searchsorted(dd, np.arange(nblk + 1) * P)
    for b in range(nblk):
        lo, hi = bounds[b], bounds[b + 1]
        s_b, d_b = ss[lo:hi], dd[lo:hi]
        a_mask = s_b < SPLIT
        blk_edges[b] = (
            (s_b[a_mask], d_b[a_mask] - b * P),
            (s_b[~a_mask] - SPLIT, d_b[~a_mask] - b * P),
        )

    # static per-slot tile counts (max over cores)
    TA, TB = [], []
    for s in range(slots):
        mxa = mxb = 0
        for c in range(NCORES):
            b = c * slots + s
            if b < nblk:
                mxa = max(mxa, len(blk_edges[b][0][0]))
                mxb = max(mxb, len(blk_edges[b][1][0]))
        ta = max(1, math.ceil(mxa / P))  # >=1 so PSUM is always written
        tb = math.ceil(mxb / P)
        TA.append(ta)
        TB.append(tb)

    # call descriptors: (slot, group, tile_offset_in_slot, ntiles, call_idx)
    # HW cap: a single dma_gather crashes beyond 1024 indices -> <=8 tiles
    MAX_NT = 8
    calls = []
    ttot = 0
    tile_off = []  # per slot, global tile offset
    for s in range(slots):
        tile_off.append(ttot)
        for grp, t0, T in ((0, 0, TA[s]), (1, TA[s], TB[s])):
            off = 0
            while off < T:
                nt = min(MAX_NT, T - off)
                calls.append((s, grp, t0 + off, nt, len(calls)))
                off += nt
        ttot += TA[s] + TB[s]
    ncalls = len(calls)

    # per-core buffers (laid out per (slot, group); gather-call chunking
    # slices this layout at tile boundaries, which lines up exactly)
    Lg = ttot * (P // 16)
    gidx = np.full((NCORES, 16, Lg), -1, np.int16)
    dstloc = np.full((NCORES, P, ttot), PAD_DST, np.float32)
    nvalid = np.full((NCORES, 1, max(ncalls, 1)), 0, np.int32)
    for c in range(NCORES):
        for s in range(slots):
            b = c * slots + s
            for grp, t0, T in ((0, 0, TA[s]), (1, TA[s], TB[s])):
                if T == 0:
                    continue
                idx_arr = np.zeros(T * P, np.int64)  # pad rows gather row 0
                if b < nblk:
                    sg, dg = blk_edges[b][grp]
                else:
                    sg = dg = np.zeros(0, np.int64)
                n = len(sg)
                assert n <= T * P
                if n:
                    idx_arr[:n] = sg
                    g0 = tile_off[s] + t0
                    pos = np.arange(n)
                    dstloc[c, pos % P, g0 + pos // P] = dg
                col0 = (tile_off[s] + t0) * (P // 16)
                gidx[c, :, col0:col0 + T * (P // 16)] = (
                    idx_arr.reshape(T * (P // 16), 16).T.astype(np.int16)
                )

    slot_tiles = [(TA[s], TB[s]) for s in range(slots)]
    plan = dict(calls=calls, slot_tiles=slot_tiles, tile_off=tile_off,
                ttot=ttot, ncalls=ncalls, Lg=Lg)
    data = dict(
        gidx=np.tile(gidx, (1, 8, 1)),          # [NC, 128, Lg]
        dstloc=dstloc,                           # [NC, 128, ttot] f32
        nvalid=nvalid,                           # [NC, 1, ncalls]
    )
    return plan, data


# ---------------------------------------------------------------------------
# Bass program builders
# ---------------------------------------------------------------------------

def build_A0(cfg):
    """Distributed phase-0 of layer 1: t1s = [h1 | as1 | ad1] for own nodes.
    All-f32 compute; h stored bf16, as/ad stored f32 (bitcast cols)."""
    NPC, IN, D1, H1 = cfg["NPC"], cfg["IN"], cfg["D1"], cfg["H1"]
    OC = D1 + 4 * H1  # out row in bf16 elems: D1 bf16 + 2*H1 f32
    nc = bacc.Bacc("TRN2", target_bir_lowering=False, debug=False)
    xs = nc.declare_dram_parameter("xs", [NPC, IN], F32, isOutput=False)
    W1 = nc.declare_dram_parameter("W1", [IN, D1], F32, isOutput=False)
    AA1 = nc.declare_dram_parameter("AA1", [D1, 2 * H1], F32, isOutput=False)
    t1s = nc.declare_dram_parameter("t1s", [NPC, OC], BF, isOutput=True)

    with tile.TileContext(nc) as tc:
        with tc.tile_pool(name="const", bufs=1) as cp, \
             tc.tile_pool(name="work", bufs=3) as wp, \
             tc.tile_pool(name="psum", bufs=1, space="PSUM") as pp:
            ident = cp.tile([P, P], F32, tag="ident")
            make_identity(nc, ident[:])
            w1 = cp.tile([IN, D1], F32, tag="w1")
            nc.sync.dma_start(out=w1[:], in_=W1[:])
            aa1 = cp.tile([D1, 2 * H1], F32, tag="aa1")
            nc.sync.dma_start(out=aa1[:], in_=AA1[:])

            for nt in range(cfg["SLOTS"]):
                rows = slice(nt * P, (nt + 1) * P)
                xt = wp.tile([P, IN], F32, tag="xt")
                nc.sync.dma_start(out=xt[:], in_=xs[rows, :])
                xTp = pp.tile([P, P], F32, tag="xTp")
                nc.tensor.transpose(out=xTp[:], in_=xt[:], identity=ident[:])
                xT = wp.tile([P, P], F32, tag="xT")
                nc.vector.tensor_copy(out=xT[:], in_=xTp[:])
                hTp = pp.tile([P, P], F32, tag="hTp")
                nc.tensor.matmul(out=hTp[:], lhsT=w1[:], rhs=xT[:],
                                 start=True, stop=True)
                hT = wp.tile([P, P], F32, tag="hT")
                nc.vector.tensor_copy(out=hT[:], in_=hTp[:])
                aaTp = pp.tile([2 * H1, P], F32, tag="aaTp")
                nc.tensor.matmul(out=aaTp[:], lhsT=aa1[:], rhs=hT[:],
                                 start=True, stop=True)
                aaT = wp.tile([2 * H1, P], F32, tag="aaT")
                nc.scalar.copy(out=aaT[:], in_=aaTp[:])
                hp = pp.tile([P, P], F32, tag="hp")
                nc.tensor.transpose(out=hp[:], in_=hT[:], identity=ident[:])
                aap = pp.tile([P, 2 * H1], F32, tag="aap")
                nc.tensor.matmul(out=aap[:], lhsT=aaT[:],
                                 rhs=ident[0:2 * H1, 0:2 * H1],
                                 start=True, stop=True)
                stage = wp.tile([P, OC], BF, tag="stage")
                nc.vector.tensor_copy(out=stage[:, 0:D1], in_=hp[:])
                nc.scalar.copy(
                    out=stage[:, D1:OC].bitcast(F32), in_=aap[:])
                nc.sync.dma_start(out=t1s[rows, :], in_=stage[:])
    nc.compile()
    return nc


def _edge_phase(nc, tc, cfg, plan, layer, T_dram, adown, gidx_d, dstloc_d,
                nvalid_d, bbc_d, out_dram, iota_d, dbg_d=None):
    """Shared edge phase. layer=1: ELU epilogue -> out_dram [NPC, D1] bf16.
    layer=2: head-mean epilogue -> out_dram [NPC, C2] f32."""
    H = cfg["H1"] if layer == 1 else cfg["H2"]
    HC = cfg["D1"] if layer == 1 else cfg["D2"]
    ROW = cfg["ROW1"] if layer == 1 else cfg["ROW2"]
    NPAD = cfg["NPAD"]
    slots = cfg["SLOTS"]
    ttot, ncalls, Lg = plan["ttot"], plan["ncalls"], plan["Lg"]
    Tmax = max(a + b for a, b in plan["slot_tiles"])

    with tc.tile_pool(name="ec", bufs=1) as cp, \
         tc.tile_pool(name="gb", bufs=2) as gp, \
         tc.tile_pool(name="ew", bufs=3) as wp, \
         tc.tile_pool(name="es", bufs=3) as sp, \
         tc.tile_pool(name="eps", bufs=2, space="PSUM") as pp, \
         tc.tile_pool(name="eacc", bufs=2, space="PSUM") as ap:
        identb = cp.tile([P, P], BF, tag="identb")
        make_identity(nc, identb[:])
        iota = cp.tile([P, P], BF, tag="iota")
        nc.sync.dma_start(out=iota[:], in_=iota_d[:])
        gidx = cp.tile([P, Lg], I16, tag="gidx")
        nc.sync.dma_start(out=gidx[:], in_=gidx_d[:])
        dsl = cp.tile([P, ttot], F32, tag="dsl")
        nc.sync.dma_start(out=dsl[:], in_=dstloc_d[:])
        ado = cp.tile([P, slots * H], F32, tag="ado")
        nc.sync.dma_start(out=ado[:], in_=adown[:])
        bbc = cp.tile([P, bbc_d.shape[1]], F32, tag="bbc")
        nc.sync.dma_start(out=bbc[:], in_=bbc_d[:])

        calls_by_slot = {}
        for (s, grp, toff, nt, ci) in plan["calls"]:
            calls_by_slot.setdefault(s, []).append((grp, toff, nt, ci))

        for s in range(slots):
            ta, tb = plan["slot_tiles"][s]
            T_s = ta + tb
            g0 = plan["tile_off"][s]
            gb = gp.tile([P, Tmax, ROW], BF, tag="gb")
            for (grp, toff, nt, ci) in calls_by_slot[s]:
                src_tab = T_dram[0:min(SPLIT, NPAD), :] if grp == 0 \
                    else T_dram[SPLIT:NPAD, :]
                nc.gpsimd.dma_gather(
                    out_ap=gb[:, toff:toff + nt, :],
                    in_ap=src_tab,
                    idxs_ap=gidx[:, (g0 + toff) * 8:(g0 + toff + nt) * 8],
                    num_idxs=nt * P,
                    num_idxs_reg=nt * P,
                    elem_size=ROW,
                )
            adb = sp.tile([P, H], BF, tag="adb")
            nc.vector.tensor_copy(out=adb[:], in_=ado[:, s * H:(s + 1) * H])
            acc = ap.tile([P, HC + H], F32, tag="acc")
            for t in range(T_s):
                S = sp.tile([P, P], BF, tag="S")
                nc.vector.tensor_scalar(
                    out=S[:], in0=iota[:], scalar1=dsl[:, g0 + t:g0 + t + 1],
                    scalar2=None, op0=mybir.AluOpType.is_equal)
                STp = pp.tile([P, P], BF, tag="STp")
                nc.tensor.transpose(out=STp[:], in_=S[:], identity=identb[:])
                ST = sp.tile([P, P], BF, tag="ST")
                nc.scalar.copy(out=ST[:], in_=STp[:])
                uE = pp.tile([P, H], F32, tag="uE")
                nc.tensor.matmul(out=uE[:], lhsT=ST[:], rhs=adb[:],
                                 start=True, stop=True)
                u = sp.tile([P, H], F32, tag="u")
                nc.vector.tensor_tensor(
                    out=u[:], in0=uE[:],
                    in1=gb[:, t, HC:HC + 2 * H].bitcast(F32),
                    op=mybir.AluOpType.add)
                lr = sp.tile([P, H], F32, tag="lr")
                nc.vector.scalar_tensor_tensor(
                    out=lr[:], in0=u[:], scalar=NEG_SLOPE, in1=u[:],
                    op0=mybir.AluOpType.mult, op1=mybir.AluOpType.max)
                Mp = sp.tile([P, HC + H], BF, tag="Mp")
                nc.scalar.activation(out=Mp[:, HC:HC + H], in_=lr[:],
                                     func=mybir.ActivationFunctionType.Exp)
                nc.vector.tensor_tensor(
                    out=Mp[:, 0:HC].rearrange("p (h c) -> p h c", h=H),
                    in0=gb[:, t, 0:HC].rearrange("p (h c) -> p h c", h=H),
                    in1=Mp[:, HC:HC + H].to_broadcast([P, H, HC // H]),
                    op=mybir.AluOpType.mult)
                nc.tensor.matmul(out=acc[:], lhsT=S[:], rhs=Mp[:],
                                 start=(t == 0), stop=(t == T_s - 1))
            # epilogue
            rows = slice(s * P, (s + 1) * P)
            if dbg_d is not None:
                dbg = wp.tile([P, HC + H], F32, tag="dbg")
                nc.vector.tensor_copy(out=dbg[:], in_=acc[:])
                nc.sync.dma_start(
                    out=dbg_d[:, s * (HC + H):(s + 1) * (HC + H)], in_=dbg[:])
            rs = wp.tile([P, H], F32, tag="rs")
            nc.vector.reciprocal(out=rs[:], in_=acc[:, HC:HC + H])
            if layer == 1:
                on = wp.tile([P, HC], F32, tag="on")
                nc.vector.tensor_tensor(
                    out=on[:].rearrange("p (h c) -> p h c", h=H),
                    in0=acc[:, 0:HC].rearrange("p (h c) -> p h c", h=H),
                    in1=rs[:].to_broadcast([P, H, HC // H]),
                    op=mybir.AluOpType.mult)
                ob = wp.tile([P, HC], F32, tag="ob")
                nc.vector.tensor_tensor(out=ob[:], in0=on[:], in1=bbc[:],
                                        op=mybir.AluOpType.add)
                # ELU = relu(x) + exp(min(x,0)) - 1
                tmin = wp.tile([P, HC], F32, tag="tmin")
                nc.vector.tensor_scalar_min(out=tmin[:], in0=ob[:],
                                            scalar1=0.0)
                ex = wp.tile([P, HC], F32, tag="ex")
                nc.scalar.activation(out=ex[:], in_=tmin[:],
                                     func=mybir.ActivationFunctionType.Exp)
                rl = wp.tile([P, HC], F32, tag="rl")
                nc.vector.tensor_scalar_max(out=rl[:], in0=ob[:],
                                            scalar1=0.0)
                stage = wp.tile([P, HC], BF, tag="stage1")
                nc.vector.scalar_tensor_tensor(
                    out=stage[:], in0=ex[:], scalar=-1.0, in1=rl[:],
                    op0=mybir.AluOpType.add, op1=mybir.AluOpType.add)
                nc.sync.dma_start(out=out_dram[rows, :], in_=stage[:])
            else:
                C2 = cfg["C2"]
                rs8 = wp.tile([P, H], F32, tag="rs8")
                nc.vector.tensor_scalar_mul(out=rs8[:], in0=rs[:],
                                            scalar1=1.0 / H)
                on = wp.tile([P, HC], F32, tag="on")
                nc.vector.tensor_tensor(
                    out=on[:].rearrange("p (h c) -> p h c", h=H),
                    in0=acc[:, 0:HC].rearrange("p (h c) -> p h c", h=H),
                    in1=rs8[:].to_broadcast([P, H, C2]),
                    op=mybir.AluOpType.mult)
                red = wp.tile([P, C2], F32, tag="red")
                nc.vector.reduce_sum(
                    out=red[:],
                    in_=on[:].rearrange("p (h c) -> p c h", h=H),
                    axis=mybir.AxisListType.X)
                stage = wp.tile([P, C2], F32, tag="stage2")
                nc.vector.tensor_tensor(out=stage[:], in0=red[:], in1=bbc[:],
                                        op=mybir.AluOpType.add)
                nc.sync.dma_start(out=out_dram[rows, :], in_=stage[:])


def build_A1(cfg, plan, debug_acc=False):
    """Layer-1 edge phase (-> h1' own slice) + distributed phase-0 of
    layer 2 (-> t2s = [h2 | as2 | ad2] own slice)."""
    NPC, NPAD, D1, D2 = cfg["NPC"], cfg["NPAD"], cfg["D1"], cfg["D2"]
    H1, H2, ROW1 = cfg["H1"], cfg["H2"], cfg["ROW1"]
    slots = cfg["SLOTS"]
    OC2 = D2 + 4 * H2
    nc = bacc.Bacc("TRN2", target_bir_lowering=False, debug=False)
    T1 = nc.declare_dram_parameter("T1", [NPAD, ROW1], BF, isOutput=False)
    ad1 = nc.declare_dram_parameter("ad1", [P, slots * H1], F32,
                                    isOutput=False)
    gx = nc.declare_dram_parameter("gidx", [P, plan["Lg"]], I16,
                                   isOutput=False)
    dl = nc.declare_dram_parameter("dstloc", [P, plan["ttot"]], F32,
                                   isOutput=False)
    nv = nc.declare_dram_parameter("nvalid", [1, plan["ncalls"]], I32,
                                   isOutput=False)
    b1 = nc.declare_dram_parameter("b1bc", [P, D1], F32, isOutput=False)
    io = nc.declare_dram_parameter("iota", [P, P], BF, isOutput=False)
    W2 = nc.declare_dram_parameter("W2", [D1, D2], BF, isOutput=False)
    AA2 = nc.declare_dram_parameter("AA2", [P, (D2 // P) * 2 * H2], BF,
                                    isOutput=False)
    h1s = nc.declare_dram_parameter("h1s", [NPC, D1], BF, isOutput=True)
    t2s = nc.declare_dram_parameter("t2s", [NPC, OC2], BF, isOutput=True)
    dbg = nc.declare_dram_parameter(
        "dbg", [P, slots * (D1 + H1)], F32, isOutput=True) if debug_acc \
        else None

    with tile.TileContext(nc) as tc:
        _edge_phase(nc, tc, cfg, plan, 1, T1, ad1, gx, dl, nv, b1, h1s, io,
                    dbg_d=dbg)
        # phase-0 of layer 2 on own h1' slice
        with tc.tile_pool(name="p0c", bufs=1) as cp, \
             tc.tile_pool(name="p0w", bufs=3) as wp, \
             tc.tile_pool(name="p0p", bufs=1, space="PSUM") as pp:
            identb = cp.tile([P, P], BF, tag="identb2")
            make_identity(nc, identb[:])
            w2 = cp.tile([D1, D2], BF, tag="w2")
            nc.sync.dma_start(out=w2[:], in_=W2[:])
            nchunk = D2 // P
            aa2 = cp.tile([P, nchunk * 2 * H2], BF, tag="aa2")
            nc.sync.dma_start(out=aa2[:], in_=AA2[:])
            for nt in range(slots):
                rows = slice(nt * P, (nt + 1) * P)
                h1T = wp.tile([P, P], BF, tag="h1T")
                nc.sync.dma_start_transpose(out=h1T[:], in_=h1s[rows, :])
                h2T = []
                for k in range(nchunk):
                    h2Tp = pp.tile([P, P], F32, tag=f"h2Tp{k}")
                    nc.tensor.matmul(out=h2Tp[:],
                                     lhsT=w2[:, k * P:(k + 1) * P],
                                     rhs=h1T[:], start=True, stop=True)
                    h2Tk = wp.tile([P, P], BF, tag=f"h2T{k}")
                    nc.vector.tensor_copy(out=h2Tk[:], in_=h2Tp[:])
                    h2T.append(h2Tk)
                aaTp = pp.tile([2 * H2, P], F32, tag="aaTp2")
                for k in range(nchunk):
                    nc.tensor.matmul(out=aaTp[:],
                                     lhsT=aa2[:, k * 2 * H2:(k + 1) * 2 * H2],
                                     rhs=h2T[k][:],
                                     start=(k == 0), stop=(k == nchunk - 1))
                aaT = wp.tile([2 * H2, P], BF, tag="aaT2")
                nc.scalar.copy(out=aaT[:], in_=aaTp[:])
                aap = pp.tile([P, 2 * H2], F32, tag="aap2")
                nc.tensor.matmul(out=aap[:], lhsT=aaT[:],
                                 rhs=identb[0:2 * H2, 0:2 * H2],
                                 start=True, stop=True)
                stage = wp.tile([P, OC2], BF, tag="stage0b")
                for k in range(nchunk):
                    hp = pp.tile([P, P], BF, tag=f"hp2{k}")
                    nc.tensor.transpose(out=hp[:], in_=h2T[k][:],
                                        identity=identb[:])
                    nc.vector.tensor_copy(out=stage[:, k * P:(k + 1) * P],
                                          in_=hp[:])
                nc.scalar.copy(out=stage[:, D2:OC2].bitcast(F32),
                               in_=aap[:])
                nc.sync.dma_start(out=t2s[rows, :], in_=stage[:])
    nc.compile()
    return nc


def build_B1(cfg, plan):
    NPC, NPAD, H2, C2 = cfg["NPC"], cfg["NPAD"], cfg["H2"], cfg["C2"]
    ROW2, slots = cfg["ROW2"], cfg["SLOTS"]
    nc = bacc.Bacc("TRN2", target_bir_lowering=False, debug=False)
    T2 = nc.declare_dram_parameter("T2", [NPAD, ROW2], BF, isOutput=False)
    ad2 = nc.declare_dram_parameter("ad2", [P, slots * H2], F32,
                                    isOutput=False)
    gx = nc.declare_dram_parameter("gidx", [P, plan["Lg"]], I16,
                                   isOutput=False)
    dl = nc.declare_dram_parameter("dstloc", [P, plan["ttot"]], F32,
                                   isOutput=False)
    nv = nc.declare_dram_parameter("nvalid", [1, plan["ncalls"]], I32,
                                   isOutput=False)
    b2 = nc.declare_dram_parameter("b2bc", [P, C2], F32, isOutput=False)
    io = nc.declare_dram_parameter("iota", [P, P], BF, isOutput=False)
    out2 = nc.declare_dram_parameter("out2", [NPC, C2], F32, isOutput=True)
    with tile.TileContext(nc) as tc:
        _edge_phase(nc, tc, cfg, plan, 2, T2, ad2, gx, dl, nv, b2, out2, io)
    nc.compile()
    return nc


# ---------------------------------------------------------------------------
# Host orchestration
# ---------------------------------------------------------------------------

def _block_diag_att(att):
    """att [H, C] -> [H*C, H] block diagonal."""
    H, C = att.shape
    out = np.zeros((H * C, H), np.float32)
    for h in range(H):
        out[h * C:(h + 1) * C, h] = att[h]
    return out


_CACHE = {}


def _get_programs(cfg, plan):
    key = (cfg["N"], cfg["E"], tuple(plan["slot_tiles"]), plan["ncalls"])
    if key not in _CACHE:
        _CACHE[key] = (build_A0(cfg), build_A1(cfg, plan),
                       build_B1(cfg, plan))
    return _CACHE[key]


def _run(nc, in_maps, **kw):
    res = run_bass_kernel_spmd(nc, in_maps, list(range(NCORES)), **kw)
    return res


def _run_timed(nc, in_maps, n_iters=3):
    """Like bass2jax.run_bass_via_pjrt but with device-resident inputs and
    repeated timed executes (min wall over n_iters after warmup)."""
    import time
    import jax
    from jax.sharding import Mesh, PartitionSpec, NamedSharding
    from jax.experimental.shard_map import shard_map
    from concourse import bass2jax, mybir as mb
    from concourse.bass2jax import _bass_exec_p, partition_id_tensor, \
        install_neuronx_cc_hook

    install_neuronx_cc_hook()
    n_cores = len(in_maps)
    partition_name = nc.partition_id_tensor.name if nc.partition_id_tensor \
        else None
    in_names, out_names, out_avals, zero_outs = [], [], [], []
    for alloc in nc.m.functions[0].allocations:
        if not isinstance(alloc, mybir.MemoryLocationSet):
            continue
        name = alloc.memorylocations[0].name
        if alloc.kind == "ExternalInput":
            if name != partition_name:
                in_names.append(name)
        elif alloc.kind == "ExternalOutput":
            shape = tuple(alloc.tensor_shape)
            dtype = mybir.dt.np(alloc.dtype)
            out_names.append(name)
            out_avals.append(jax.core.ShapedArray(shape, dtype))
            zero_outs.append(np.zeros(shape, dtype))
    n_params = len(in_names)
    n_outs = len(out_avals)
    in_names_all = in_names + out_names
    if partition_name is not None:
        in_names_all = in_names_all + [partition_name]

    def _body(*args):
        operands = list(args)
        if partition_name is not None:
            operands.append(partition_id_tensor())
        return tuple(_bass_exec_p.bind(
            *operands, out_avals=tuple(out_avals),
            in_names=tuple(in_names_all), out_names=tuple(out_names),
            lowering_input_output_aliases=(),
            sim_require_finite=True, sim_require_nnan=True, nc=nc))

    devices = jax.devices()[:n_cores]
    mesh = Mesh(np.asarray(devices), ("core",))
    spec = PartitionSpec("core")
    sharded = jax.jit(
        shard_map(_body, mesh=mesh, in_specs=(spec,) * (n_params + n_outs),
                  out_specs=(spec,) * n_outs, check_rep=False),
        keep_unused=True)
    sh = NamedSharding(mesh, spec)
    dev_in = [
        jax.device_put(
            np.concatenate([np.asarray(in_maps[c][nm]) for c in
                            range(n_cores)], axis=0), sh)
        for nm in in_names
    ]
    dev_zero = [
        jax.device_put(
            np.zeros((n_cores * z.shape[0], *z.shape[1:]), z.dtype), sh)
        for z in zero_outs
    ]
    out = sharded(*dev_in, *dev_zero)  # warmup + compile
    jax.block_until_ready(out)
    wall = []
    for _ in range(n_iters):
        t0 = time.perf_counter()
        o = sharded(*dev_in, *dev_zero)
        jax.block_until_ready(o)
        wall.append(time.perf_counter() - t0)
    results = [
        {nm: np.asarray(out[i]).reshape(n_cores, *out_avals[i].shape)[c]
         for i, nm in enumerate(out_names)}
        for c in range(n_cores)
    ]

    class R:
        pass
    r = R()
    r.results = results
    r.exec_time_ns = int(min(wall) * 1e9)
    r.wall_all = wall
    return r


def kernel(x, edge_index, W1, att_src1, att_dst1, b1, W2, att_src2,
           att_dst2, b2, _collect_times=None, _cfg_override=None,
           _runner=None):
    cfg = _cfg_override or CFG
    N, NPC, NPAD = cfg["N"], cfg["NPC"], cfg["NPAD"]
    D1, D2, H1, H2, C2 = cfg["D1"], cfg["D2"], cfg["H1"], cfg["H2"], cfg["C2"]
    ROW1, ROW2, slots = cfg["ROW1"], cfg["ROW2"], cfg["SLOTS"]

    x = np.asarray(x, np.float32)
    ei = np.asarray(edge_index)
    loops = np.arange(N, dtype=ei.dtype)
    src_n = np.concatenate([ei[0], loops])
    dst_n = np.concatenate([ei[1], loops])

    plan, edata = build_edge_plan(cfg, src_n, dst_n)
    ncA0, ncA1, ncB1 = _get_programs(cfg, plan)
    if _runner is not None:
        run = _runner
    elif _collect_times is not None:
        run = _run_timed
    else:
        run = _run

    # ---- launch A0: per-node h1/as1/ad1 ----
    xpad = np.zeros((NCORES * NPC, cfg["IN"]), np.float32)
    xpad[:N] = x
    AA1 = np.concatenate([_block_diag_att(np.asarray(att_src1, np.float32)),
                          _block_diag_att(np.asarray(att_dst1, np.float32))],
                         axis=1)
    in_maps = [
        dict(xs=xpad[c * NPC:(c + 1) * NPC],
             W1=np.asarray(W1, np.float32), AA1=AA1)
        for c in range(NCORES)
    ]
    resA0 = run(ncA0, in_maps)
    t1s = [resA0.results[c]["t1s"] for c in range(NCORES)]
    if _collect_times is not None:
        _collect_times.append(("A0", resA0.exec_time_ns))

    # assemble replicated T1 [NPAD, ROW1]
    OC1 = D1 + 4 * H1
    T1 = np.zeros((NPAD, ROW1), BF16)
    full = np.concatenate(t1s, axis=0)  # [8*NPC, OC1]
    T1[:, :OC1] = full[:NPAD]
    ad1own = np.stack([
        np.ascontiguousarray(t1s[c][:, D1 + 2 * H1:OC1]).view(np.float32)
        .reshape(slots, P, H1).transpose(1, 0, 2).reshape(P, slots * H1)
        for c in range(NCORES)
    ])

    # ---- launch A1: layer-1 edges + phase0 of layer 2 ----
    AA2 = np.concatenate([_block_diag_att(np.asarray(att_src2, np.float32)),
                          _block_diag_att(np.asarray(att_dst2, np.float32))],
                         axis=1)
    b1bc = np.tile(np.asarray(b1, np.float32)[None, :], (P, 1))
    in_maps = [
        dict(T1=T1, ad1=np.ascontiguousarray(ad1own[c]),
             gidx=edata["gidx"][c], dstloc=edata["dstloc"][c],
             nvalid=edata["nvalid"][c], b1bc=b1bc, iota=IOTA,
             W2=np.asarray(W2, np.float32).astype(BF16),
             AA2=np.concatenate(
                 [AA2[k * P:(k + 1) * P] for k in range(D2 // P)],
                 axis=1).astype(BF16))
        for c in range(NCORES)
    ]
    resA1 = run(ncA1, in_maps)
    t2s = [resA1.results[c]["t2s"] for c in range(NCORES)]
    if _collect_times is not None:
        _collect_times.append(("A1", resA1.exec_time_ns))

    OC2 = D2 + 4 * H2
    T2 = np.zeros((NPAD, ROW2), BF16)
    full2 = np.concatenate(t2s, axis=0)
    T2[:, :OC2] = full2[:NPAD]
    ad2own = np.stack([
        np.ascontiguousarray(t2s[c][:, D2 + 2 * H2:OC2]).view(np.float32)
        .reshape(slots, P, H2).transpose(1, 0, 2).reshape(P, slots * H2)
        for c in range(NCORES)
    ])

    # ---- launch B1: layer-2 edges ----
    b2bc = np.tile(np.asarray(b2, np.float32)[None, :], (P, 1))
    in_maps = [
        dict(T2=T2, ad2=np.ascontiguousarray(ad2own[c]),
             gidx=edata["gidx"][c], dstloc=edata["dstloc"][c],
             nvalid=edata["nvalid"][c], b2bc=b2bc, iota=IOTA)
        for c in range(NCORES)
    ]
    resB1 = run(ncB1, in_maps)
    if _collect_times is not None:
        _collect_times.append(("B1", resB1.exec_time_ns))
    out = np.concatenate([resB1.results[c]["out2"] for c in range(NCORES)],
                         axis=0)[:N]
    return np.asarray(out, np.float32)



# revision 15
# speedup vs baseline: 2.8333x; 2.8333x over previous
"""GAT (2-layer, PyG-style) Trainium2 Bass kernel, 8-core SPMD, fused.

Strategy (edge parallelism by destination):
  - Add self loops, sort edges by dst, partition dst-node blocks of 128
    across 8 cores (contiguous block ranges).
  - ONE launch. Per layer, each core computes its own slice of the
    node-feature table T (row-per-node: [h bf16 | a_src f32], 256B-multiple
    row stride), an on-device AllGather replicates T to every core, then
    the edge phase gathers T[src] rows per dst block.
  - Edge phase per core: for each of its dst blocks, batched dma_gather
    of T[src] rows (int16 idx limit 32767 -> two gathers split by
    src < 32768), then per 128-edge tile:
      S[e,d] = (dstloc_e == d)                 (DVE tensor_scalar is_equal)
      ST = S^T                                 (PE transpose)
      u = ST.T @ a_dst_block + a_src_gathered  (PE matmul + DVE add)
      p = exp(leaky_relu(u))                   (ACT Lrelu, ACT Exp)
      M = h_gathered * p (per-head broadcast)  (DVE)
      acc[d, :] += S.T @ [M | p]               (PE matmul, PSUM accumulate)
    Segment softmax without max-subtraction (logits are O(10), exact in
    f32: softmax is shift-invariant so this matches the reference).
  - Block epilogue: out = acc[:, :HC] / acc[:, HC:] (per head), + bias,
    ELU (layer 1) or head-mean (layer 2).
  - a_dst values for a core's own dst blocks never travel through the
    table: phase-0 writes them into a persistent SBUF tile directly.

Program layout (single Bass program):
  A0:  x_slice @ W1 -> t1 slice rows [h1|as1] + ad1 SBUF     (distributed)
  CC1: AllGather t1 slice -> full T1 (DRAM)
  E1:  layer-1 edge phase -> h1' slice (DRAM);
       h1' @ W2 -> t2 slice rows [h2|as2] + ad2 SBUF
  CC2: AllGather t2 slice -> full T2 (DRAM)
  E2:  layer-2 edge phase -> out slice (ExternalOutput)
"""

import sys

sys.path.insert(0, "/opt/trn_rl_repo")

import math
import numpy as np
import ml_dtypes

import concourse.bass as bass
import concourse.bacc as bacc
import concourse.tile as tile
from concourse import mybir
from concourse.bass_utils import run_bass_kernel_spmd
from concourse.masks import make_identity

BF16 = ml_dtypes.bfloat16
F32 = mybir.dt.float32
BF = mybir.dt.bfloat16
I16 = mybir.dt.int16
I32 = mybir.dt.int32

P = 128
NCORES = 8
SPLIT = 32768
NEG_SLOPE = 0.2
PAD_DST = 1000.0  # dstloc sentinel: matches no d in [0,128)
IOTA = np.tile(np.arange(P, dtype=np.float32), (P, 1)).astype(ml_dtypes.bfloat16)


def _cfg(N, E, IN, H1, C1, H2, C2):
    nblk = math.ceil(N / P)
    slots = math.ceil(nblk / NCORES)
    return dict(
        N=N, E=E, IN=IN, H1=H1, C1=C1, H2=H2, C2=C2,
        D1=H1 * C1, D2=H2 * C2,
        NBLK=nblk, SLOTS=slots, NPC=slots * P, NPAD=nblk * P,
        # table row lengths in bf16 elems (256B-multiple strides)
        ROW1=_row_elems(H1 * C1 + 2 * H1),  # h bf16 + as f32
        ROW2=_row_elems(H2 * C2 + 2 * H2),
    )


def _row_elems(used_bf16_elems):
    # round row up to a multiple of 128 bf16 elems (256 bytes)
    return ((used_bf16_elems + 127) // 128) * 128


CFG = _cfg(N=50000, E=800000, IN=128, H1=4, C1=32, H2=8, C2=32)


# ---------------------------------------------------------------------------
# Host-side edge plan
# ---------------------------------------------------------------------------

def build_edge_plan(cfg, src, dst):
    """Sort by dst, bucket into (core, slot) dst blocks, split each block's
    edges by src < SPLIT, pad each group to a multiple of 128.

    Returns a static `plan` (identical across cores: per-slot tile counts
    and call descriptors) plus per-core data buffers (gather indices,
    local-dst per tile)."""
    slots, nblk = cfg["SLOTS"], cfg["NBLK"]
    order = np.argsort(dst, kind="stable")
    ss = src[order].astype(np.int64)
    dd = dst[order].astype(np.int64)
    blk_edges = {}
    bounds = np.searchsorted(dd, np.arange(nblk + 1) * P)
    for b in range(nblk):
        lo, hi = bounds[b], bounds[b + 1]
        s_b, d_b = ss[lo:hi], dd[lo:hi]
        a_mask = s_b < SPLIT
        blk_edges[b] = (
            (s_b[a_mask], d_b[a_mask] - b * P),
            (s_b[~a_mask] - SPLIT, d_b[~a_mask] - b * P),
        )

    # static per-slot tile counts (max over cores)
    TA, TB = [], []
    for s in range(slots):
        mxa = mxb = 0
        for c in range(NCORES):
            b = c * slots + s
            if b < nblk:
                mxa = max(mxa, len(blk_edges[b][0][0]))
                mxb = max(mxb, len(blk_edges[b][1][0]))
        ta = max(1, math.ceil(mxa / P))  # >=1 so PSUM is always written
        tb = math.ceil(mxb / P)
        TA.append(ta)
        TB.append(tb)

    # call descriptors: (slot, group, tile_offset_in_slot, ntiles, call_idx)
    # HW cap: a single dma_gather crashes beyond 1024 indices -> <=8 tiles
    MAX_NT = 8
    calls = []
    ttot = 0
    tile_off = []  # per slot, global tile offset
    for s in range(slots):
        tile_off.append(ttot)
        for grp, t0, T in ((0, 0, TA[s]), (1, TA[s], TB[s])):
            off = 0
            while off < T:
                nt = min(MAX_NT, T - off)
                calls.append((s, grp, t0 + off, nt, len(calls)))
                off += nt
        ttot += TA[s] + TB[s]
    ncalls = len(calls)

    # per-core buffers (laid out per (slot, group); gather-call chunking
    # slices this layout at tile boundaries, which lines up exactly)
    Lg = ttot * (P // 16)
    gidx = np.full((NCORES, 16, Lg), -1, np.int16)
    dstloc = np.full((NCORES, P, ttot), PAD_DST, np.float32)
    for c in range(NCORES):
        for s in range(slots):
            b = c * slots + s
            for grp, t0, T in ((0, 0, TA[s]), (1, TA[s], TB[s])):
                if T == 0:
                    continue
                idx_arr = np.zeros(T * P, np.int64)  # pad rows gather row 0
                if b < nblk:
                    sg, dg = blk_edges[b][grp]
                else:
                    sg = dg = np.zeros(0, np.int64)
                n = len(sg)
                assert n <= T * P
                if n:
                    idx_arr[:n] = sg
                    g0 = tile_off[s] + t0
                    pos = np.arange(n)
                    dstloc[c, pos % P, g0 + pos // P] = dg
                col0 = (tile_off[s] + t0) * (P // 16)
                gidx[c, :, col0:col0 + T * (P // 16)] = (
                    idx_arr.reshape(T * (P // 16), 16).T.astype(np.int16)
                )

    slot_tiles = [(TA[s], TB[s]) for s in range(slots)]
    plan = dict(calls=calls, slot_tiles=slot_tiles, tile_off=tile_off,
                ttot=ttot, ncalls=ncalls, Lg=Lg)
    data = dict(
        gidx=np.tile(gidx, (1, 8, 1)),          # [NC, 128, Lg]
        dstloc=dstloc,                           # [NC, 128, ttot] f32
    )
    return plan, data


# ---------------------------------------------------------------------------
# Bass program builder (single fused program)
# ---------------------------------------------------------------------------

def _edge_phase(nc, tc, cfg, plan, layer, T_dram, ado, identb, iota,
                bbc_d, out_dram, gidx_d, dstloc_d):
    """Shared edge phase. layer=1: ELU epilogue -> out_dram [NPC, D1] bf16.
    layer=2: head-mean epilogue -> out_dram [NPC, C2] f32.
    `ado` is a persistent SBUF tile [P, slots*H] with a_dst of own nodes."""
    H = cfg["H1"] if layer == 1 else cfg["H2"]
    HC = cfg["D1"] if layer == 1 else cfg["D2"]
    ROW = cfg["ROW1"] if layer == 1 else cfg["ROW2"]
    NTAB = NCORES * cfg["NPC"]
    slots = cfg["SLOTS"]
    ttot, Lg = plan["ttot"], plan["Lg"]
    Tmax = max(a + b for a, b in plan["slot_tiles"])

    with tc.tile_pool(name=f"ec{layer}", bufs=1) as cp, \
         tc.tile_pool(name=f"gb{layer}", bufs=2) as gp, \
         tc.tile_pool(name=f"ew{layer}", bufs=3) as wp, \
         tc.tile_pool(name=f"es{layer}", bufs=3) as sp, \
         tc.tile_pool(name=f"eps{layer}", bufs=2, space="PSUM") as pp, \
         tc.tile_pool(name=f"eacc{layer}", bufs=2, space="PSUM") as ap:
        gidx = cp.tile([P, Lg], I16, tag="gidx")
        nc.sync.dma_start(out=gidx[:], in_=gidx_d[:])
        dsl = cp.tile([P, ttot], F32, tag="dsl")
        nc.sync.dma_start(out=dsl[:], in_=dstloc_d[:])
        bbc = cp.tile([P, bbc_d.shape[1]], F32, tag="bbc")
        nc.sync.dma_start(out=bbc[:], in_=bbc_d[:])

        calls_by_slot = {}
        for (s, grp, toff, nt, ci) in plan["calls"]:
            calls_by_slot.setdefault(s, []).append((grp, toff, nt, ci))

        for s in range(slots):
            ta, tb = plan["slot_tiles"][s]
            T_s = ta + tb
            g0 = plan["tile_off"][s]
            gb = gp.tile([P, Tmax, ROW], BF, tag="gb")
            for (grp, toff, nt, ci) in calls_by_slot[s]:
                src_tab = T_dram[0:min(SPLIT, NTAB), :] if grp == 0 \
                    else T_dram[SPLIT:NTAB, :]
                nc.gpsimd.dma_gather(
                    out_ap=gb[:, toff:toff + nt, :],
                    in_ap=src_tab,
                    idxs_ap=gidx[:, (g0 + toff) * 8:(g0 + toff + nt) * 8],
                    num_idxs=nt * P,
                    num_idxs_reg=nt * P,
                    elem_size=ROW,
                )
            adb = sp.tile([P, H], BF, tag="adb")
            nc.vector.tensor_copy(out=adb[:], in_=ado[:, s * H:(s + 1) * H])
            acc = ap.tile([P, HC + H], F32, tag="acc")
            for t in range(T_s):
                S = sp.tile([P, P], BF, tag="S")
                nc.vector.tensor_scalar(
                    out=S[:], in0=iota[:], scalar1=dsl[:, g0 + t:g0 + t + 1],
                    scalar2=None, op0=mybir.AluOpType.is_equal)
                STp = pp.tile([P, P], BF, tag="STp")
                nc.tensor.transpose(out=STp[:], in_=S[:], identity=identb[:])
                ST = sp.tile([P, P], BF, tag="ST")
                nc.scalar.copy(out=ST[:], in_=STp[:])
                uE = pp.tile([P, H], F32, tag="uE")
                nc.tensor.matmul(out=uE[:], lhsT=ST[:], rhs=adb[:],
                                 start=True, stop=True)
                u = sp.tile([P, H], F32, tag="u")
                nc.vector.tensor_tensor(
                    out=u[:], in0=uE[:],
                    in1=gb[:, t, HC:HC + 2 * H].bitcast(F32),
                    op=mybir.AluOpType.add)
                lr = sp.tile([P, H], F32, tag="lr")
                nc.vector.scalar_tensor_tensor(
                    out=lr[:], in0=u[:], scalar=NEG_SLOPE, in1=u[:],
                    op0=mybir.AluOpType.mult, op1=mybir.AluOpType.max)
                Mp = sp.tile([P, HC + H], BF, tag="Mp")
                nc.scalar.activation(out=Mp[:, HC:HC + H], in_=lr[:],
                                     func=mybir.ActivationFunctionType.Exp)
                nc.vector.tensor_tensor(
                    out=Mp[:, 0:HC].rearrange("p (h c) -> p h c", h=H),
                    in0=gb[:, t, 0:HC].rearrange("p (h c) -> p h c", h=H),
                    in1=Mp[:, HC:HC + H].to_broadcast([P, H, HC // H]),
                    op=mybir.AluOpType.mult)
                nc.tensor.matmul(out=acc[:], lhsT=S[:], rhs=Mp[:],
                                 start=(t == 0), stop=(t == T_s - 1))
            # epilogue
            rows = slice(s * P, (s + 1) * P)
            rs = wp.tile([P, H], F32, tag="rs")
            nc.vector.reciprocal(out=rs[:], in_=acc[:, HC:HC + H])
            if layer == 1:
                on = wp.tile([P, HC], F32, tag="on")
                nc.vector.tensor_tensor(
                    out=on[:].rearrange("p (h c) -> p h c", h=H),
                    in0=acc[:, 0:HC].rearrange("p (h c) -> p h c", h=H),
                    in1=rs[:].to_broadcast([P, H, HC // H]),
                    op=mybir.AluOpType.mult)
                ob = wp.tile([P, HC], F32, tag="ob")
                nc.vector.tensor_tensor(out=ob[:], in0=on[:], in1=bbc[:],
                                        op=mybir.AluOpType.add)
                # ELU = relu(x) + exp(min(x,0)) - 1
                tmin = wp.tile([P, HC], F32, tag="tmin")
                nc.vector.tensor_scalar_min(out=tmin[:], in0=ob[:],
                                            scalar1=0.0)
                ex = wp.tile([P, HC], F32, tag="ex")
                nc.scalar.activation(out=ex[:], in_=tmin[:],
                                     func=mybir.ActivationFunctionType.Exp)
                rl = wp.tile([P, HC], F32, tag="rl")
                nc.vector.tensor_scalar_max(out=rl[:], in0=ob[:],
                                            scalar1=0.0)
                stage = wp.tile([P, HC], BF, tag="stage1")
                nc.vector.scalar_tensor_tensor(
                    out=stage[:], in0=ex[:], scalar=-1.0, in1=rl[:],
                    op0=mybir.AluOpType.add, op1=mybir.AluOpType.add)
                nc.sync.dma_start(out=out_dram[rows, :], in_=stage[:])
            else:
                C2 = cfg["C2"]
                rs8 = wp.tile([P, H], F32, tag="rs8")
                nc.vector.tensor_scalar_mul(out=rs8[:], in0=rs[:],
                                            scalar1=1.0 / H)
                on = wp.tile([P, HC], F32, tag="on")
                nc.vector.tensor_tensor(
                    out=on[:].rearrange("p (h c) -> p h c", h=H),
                    in0=acc[:, 0:HC].rearrange("p (h c) -> p h c", h=H),
                    in1=rs8[:].to_broadcast([P, H, C2]),
                    op=mybir.AluOpType.mult)
                red = wp.tile([P, C2], F32, tag="red")
                nc.vector.reduce_sum(
                    out=red[:],
                    in_=on[:].rearrange("p (h c) -> p c h", h=H),
                    axis=mybir.AxisListType.X)
                stage = wp.tile([P, C2], F32, tag="stage2")
                nc.vector.tensor_tensor(out=stage[:], in0=red[:], in1=bbc[:],
                                        op=mybir.AluOpType.add)
                nc.sync.dma_start(out=out_dram[rows, :], in_=stage[:])


def build_fused(cfg, plan):
    NPC, IN = cfg["NPC"], cfg["IN"]
    D1, D2, H1, H2, C2 = cfg["D1"], cfg["D2"], cfg["H1"], cfg["H2"], cfg["C2"]
    ROW1, ROW2, slots = cfg["ROW1"], cfg["ROW2"], cfg["SLOTS"]
    NTAB = NCORES * NPC

    nc = bacc.Bacc("TRN2", target_bir_lowering=False, debug=False,
                   num_devices=NCORES)
    xs = nc.declare_dram_parameter("xs", [NPC, IN], F32, isOutput=False)
    W1 = nc.declare_dram_parameter("W1", [IN, D1], F32, isOutput=False)
    AA1 = nc.declare_dram_parameter("AA1", [D1, 2 * H1], F32, isOutput=False)
    b1 = nc.declare_dram_parameter("b1bc", [P, D1], F32, isOutput=False)
    W2 = nc.declare_dram_parameter("W2", [D1, D2], BF, isOutput=False)
    AA2 = nc.declare_dram_parameter("AA2", [P, (D2 // P) * 2 * H2], BF,
                                    isOutput=False)
    b2 = nc.declare_dram_parameter("b2bc", [P, C2], F32, isOutput=False)
    io = nc.declare_dram_parameter("iota", [P, P], BF, isOutput=False)
    gidx_d = nc.declare_dram_parameter("gidx", [P, plan["Lg"]], I16,
                                       isOutput=False)
    dstloc_d = nc.declare_dram_parameter("dstloc", [P, plan["ttot"]], F32,
                                         isOutput=False)
    out2 = nc.declare_dram_parameter("out2", [NPC, C2], F32, isOutput=True)

    groups = [list(range(NCORES))]

    with tile.TileContext(nc) as tc:
        with tc.tile_pool(name="dram", bufs=1, space="DRAM") as dp, \
             tc.tile_pool(name="pers", bufs=1) as pers:
            t1s_d = dp.tile([NPC, ROW1], BF, tag="t1s")
            T1full = dp.tile([NTAB, ROW1], BF, tag="T1full")
            h1_d = dp.tile([NPC, D1], BF, tag="h1d")
            t2s_d = dp.tile([NPC, ROW2], BF, tag="t2s")
            T2full = dp.tile([NTAB, ROW2], BF, tag="T2full")

            ado1 = pers.tile([P, slots * H1], F32, tag="ado1")
            ado2 = pers.tile([P, slots * H2], F32, tag="ado2")
            identf = pers.tile([P, P], F32, tag="identf")
            make_identity(nc, identf[:])
            identb = pers.tile([P, P], BF, tag="identb")
            make_identity(nc, identb[:])
            iota = pers.tile([P, P], BF, tag="iota")
            nc.sync.dma_start(out=iota[:], in_=io[:])

            # ---- phase A0: own nodes -> t1 slice rows [h1|as1], ad1 SBUF
            with tc.tile_pool(name="a0c", bufs=1) as cp, \
                 tc.tile_pool(name="a0w", bufs=3) as wp, \
                 tc.tile_pool(name="a0p", bufs=1, space="PSUM") as pp:
                w1 = cp.tile([IN, D1], F32, tag="w1")
                nc.sync.dma_start(out=w1[:], in_=W1[:])
                aa1 = cp.tile([D1, 2 * H1], F32, tag="aa1")
                nc.sync.dma_start(out=aa1[:], in_=AA1[:])
                for nt in range(slots):
                    rows = slice(nt * P, (nt + 1) * P)
                    xt = wp.tile([P, IN], F32, tag="xt")
                    nc.sync.dma_start(out=xt[:], in_=xs[rows, :])
                    xTp = pp.tile([P, P], F32, tag="xTp")
                    nc.tensor.transpose(out=xTp[:], in_=xt[:],
                                        identity=identf[:])
                    xT = wp.tile([P, P], F32, tag="xT")
                    nc.vector.tensor_copy(out=xT[:], in_=xTp[:])
                    hTp = pp.tile([P, P], F32, tag="hTp")
                    nc.tensor.matmul(out=hTp[:], lhsT=w1[:], rhs=xT[:],
                                     start=True, stop=True)
                    hT = wp.tile([P, P], F32, tag="hT")
                    nc.vector.tensor_copy(out=hT[:], in_=hTp[:])
                    aaTp = pp.tile([2 * H1, P], F32, tag="aaTp")
                    nc.tensor.matmul(out=aaTp[:], lhsT=aa1[:], rhs=hT[:],
                                     start=True, stop=True)
                    aaT = wp.tile([2 * H1, P], F32, tag="aaT")
                    nc.scalar.copy(out=aaT[:], in_=aaTp[:])
                    hp = pp.tile([P, P], F32, tag="hp")
                    nc.tensor.transpose(out=hp[:], in_=hT[:],
                                        identity=identf[:])
                    aap = pp.tile([P, 2 * H1], F32, tag="aap")
                    nc.tensor.matmul(out=aap[:], lhsT=aaT[:],
                                     rhs=identf[0:2 * H1, 0:2 * H1],
                                     start=True, stop=True)
                    stage = wp.tile([P, ROW1], BF, tag="stage")
                    nc.vector.tensor_copy(out=stage[:, 0:D1], in_=hp[:])
                    nc.scalar.copy(
                        out=stage[:, D1:D1 + 2 * H1].bitcast(F32),
                        in_=aap[:, 0:H1])
                    nc.vector.tensor_copy(
                        out=ado1[:, nt * H1:(nt + 1) * H1],
                        in_=aap[:, H1:2 * H1])
                    nc.sync.dma_start(out=t1s_d[rows, :], in_=stage[:])

            # ---- CC1: AllGather t1 slice -> full T1
            nc.gpsimd.collective_compute(
                "AllGather", mybir.AluOpType.bypass, replica_groups=groups,
                ins=[t1s_d[:].opt()], outs=[T1full[:].opt()])

            # ---- E1: layer-1 edge phase -> h1' slice
            _edge_phase(nc, tc, cfg, plan, 1, T1full, ado1, identb, iota,
                        b1, h1_d, gidx_d, dstloc_d)

            # ---- phase-0 of layer 2 on own h1' slice
            with tc.tile_pool(name="p0c", bufs=1) as cp, \
                 tc.tile_pool(name="p0w", bufs=3) as wp, \
                 tc.tile_pool(name="p0p", bufs=1, space="PSUM") as pp:
                w2 = cp.tile([D1, D2], BF, tag="w2")
                nc.sync.dma_start(out=w2[:], in_=W2[:])
                nchunk = D2 // P
                aa2 = cp.tile([P, nchunk * 2 * H2], BF, tag="aa2")
                nc.sync.dma_start(out=aa2[:], in_=AA2[:])
                for nt in range(slots):
                    rows = slice(nt * P, (nt + 1) * P)
                    h1T = wp.tile([P, P], BF, tag="h1T")
                    nc.sync.dma_start_transpose(out=h1T[:], in_=h1_d[rows, :])
                    h2T = []
                    for k in range(nchunk):
                        h2Tp = pp.tile([P, P], F32, tag=f"h2Tp{k}")
                        nc.tensor.matmul(out=h2Tp[:],
                                         lhsT=w2[:, k * P:(k + 1) * P],
                                         rhs=h1T[:], start=True, stop=True)
                        h2Tk = wp.tile([P, P], BF, tag=f"h2T{k}")
                        nc.vector.tensor_copy(out=h2Tk[:], in_=h2Tp[:])
                        h2T.append(h2Tk)
                    aaTp = pp.tile([2 * H2, P], F32, tag="aaTp2")
                    for k in range(nchunk):
                        nc.tensor.matmul(
                            out=aaTp[:],
                            lhsT=aa2[:, k * 2 * H2:(k + 1) * 2 * H2],
                            rhs=h2T[k][:],
                            start=(k == 0), stop=(k == nchunk - 1))
                    aaT = wp.tile([2 * H2, P], BF, tag="aaT2")
                    nc.scalar.copy(out=aaT[:], in_=aaTp[:])
                    aap = pp.tile([P, 2 * H2], F32, tag="aap2")
                    nc.tensor.matmul(out=aap[:], lhsT=aaT[:],
                                     rhs=identb[0:2 * H2, 0:2 * H2],
                                     start=True, stop=True)
                    stage = wp.tile([P, ROW2], BF, tag="stage0b")
                    for k in range(nchunk):
                        hp = pp.tile([P, P], BF, tag=f"hp2{k}")
                        nc.tensor.transpose(out=hp[:], in_=h2T[k][:],
                                            identity=identb[:])
                        nc.vector.tensor_copy(out=stage[:, k * P:(k + 1) * P],
                                              in_=hp[:])
                    nc.scalar.copy(
                        out=stage[:, D2:D2 + 2 * H2].bitcast(F32),
                        in_=aap[:, 0:H2])
                    nc.vector.tensor_copy(
                        out=ado2[:, nt * H2:(nt + 1) * H2],
                        in_=aap[:, H2:2 * H2])
                    nc.sync.dma_start(out=t2s_d[rows, :], in_=stage[:])

            # ---- CC2: AllGather t2 slice -> full T2
            nc.gpsimd.collective_compute(
                "AllGather", mybir.AluOpType.bypass, replica_groups=groups,
                ins=[t2s_d[:].opt()], outs=[T2full[:].opt()])

            # ---- E2: layer-2 edge phase -> out slice
            _edge_phase(nc, tc, cfg, plan, 2, T2full, ado2, identb, iota,
                        b2, out2, gidx_d, dstloc_d)
    nc.compile()
    return nc


# ---------------------------------------------------------------------------
# Host orchestration
# ---------------------------------------------------------------------------

def _block_diag_att(att):
    """att [H, C] -> [H*C, H] block diagonal."""
    H, C = att.shape
    out = np.zeros((H * C, H), np.float32)
    for h in range(H):
        out[h * C:(h + 1) * C, h] = att[h]
    return out


_CACHE = {}


def _get_program(cfg, plan):
    key = (cfg["N"], cfg["E"], tuple(plan["slot_tiles"]), plan["ncalls"])
    if key not in _CACHE:
        _CACHE[key] = build_fused(cfg, plan)
    return _CACHE[key]


def _run(nc, in_maps, **kw):
    res = run_bass_kernel_spmd(nc, in_maps, list(range(NCORES)), **kw)
    return res


def _run_timed(nc, in_maps, n_iters=3):
    """Like bass2jax.run_bass_via_pjrt but with device-resident inputs and
    repeated timed executes (min wall over n_iters after warmup)."""
    import time
    import jax
    from jax.sharding import Mesh, PartitionSpec, NamedSharding
    from jax.experimental.shard_map import shard_map
    from concourse.bass2jax import _bass_exec_p, partition_id_tensor, \
        install_neuronx_cc_hook

    install_neuronx_cc_hook()
    n_cores = len(in_maps)
    partition_name = nc.partition_id_tensor.name if nc.partition_id_tensor \
        else None
    in_names, out_names, out_avals, zero_outs = [], [], [], []
    for alloc in nc.m.functions[0].allocations:
        if not isinstance(alloc, mybir.MemoryLocationSet):
            continue
        name = alloc.memorylocations[0].name
        if alloc.kind == "ExternalInput":
            if name != partition_name:
                in_names.append(name)
        elif alloc.kind == "ExternalOutput":
            shape = tuple(alloc.tensor_shape)
            dtype = mybir.dt.np(alloc.dtype)
            out_names.append(name)
            out_avals.append(jax.core.ShapedArray(shape, dtype))
            zero_outs.append(np.zeros(shape, dtype))
    n_params = len(in_names)
    n_outs = len(out_avals)
    in_names_all = in_names + out_names
    if partition_name is not None:
        in_names_all = in_names_all + [partition_name]

    def _body(*args):
        operands = list(args)
        if partition_name is not None:
            operands.append(partition_id_tensor())
        return tuple(_bass_exec_p.bind(
            *operands, out_avals=tuple(out_avals),
            in_names=tuple(in_names_all), out_names=tuple(out_names),
            lowering_input_output_aliases=(),
            sim_require_finite=True, sim_require_nnan=True, nc=nc))

    devices = jax.devices()[:n_cores]
    mesh = Mesh(np.asarray(devices), ("core",))
    spec = PartitionSpec("core")
    # Donate the zero output buffers: NEFFs with collectives depend on the
    # donation mechanism (outputs must alias the pre-zeroed operands).
    donate = tuple(range(n_params, n_params + n_outs))
    sharded = jax.jit(
        shard_map(_body, mesh=mesh, in_specs=(spec,) * (n_params + n_outs),
                  out_specs=(spec,) * n_outs, check_rep=False),
        donate_argnums=donate, keep_unused=True)
    sh = NamedSharding(mesh, spec)
    dev_in = [
        jax.device_put(
            np.concatenate([np.asarray(in_maps[c][nm]) for c in
                            range(n_cores)], axis=0), sh)
        for nm in in_names
    ]
    host_zeros = [
        np.zeros((n_cores * z.shape[0], *z.shape[1:]), z.dtype)
        for z in zero_outs
    ]

    def _fresh_zeros():
        dz = [jax.device_put(z, sh) for z in host_zeros]
        jax.block_until_ready(dz)
        return dz

    out = sharded(*dev_in, *_fresh_zeros())  # warmup + compile
    jax.block_until_ready(out)
    wall = []
    for _ in range(n_iters):
        dz = _fresh_zeros()
        t0 = time.perf_counter()
        o = sharded(*dev_in, *dz)
        jax.block_until_ready(o)
        wall.append(time.perf_counter() - t0)
    results = [
        {nm: np.asarray(out[i]).reshape(n_cores, *out_avals[i].shape)[c]
         for i, nm in enumerate(out_names)}
        for c in range(n_cores)
    ]

    class R:
        pass
    r = R()
    r.results = results
    r.exec_time_ns = int(min(wall) * 1e9)
    r.wall_all = wall
    return r


def kernel(x, edge_index, W1, att_src1, att_dst1, b1, W2, att_src2,
           att_dst2, b2, _collect_times=None, _cfg_override=None,
           _runner=None):
    cfg = _cfg_override or CFG
    N, NPC = cfg["N"], cfg["NPC"]
    D2, H2 = cfg["D2"], cfg["H2"]

    x = np.asarray(x, np.float32)
    ei = np.asarray(edge_index)
    loops = np.arange(N, dtype=ei.dtype)
    src_n = np.concatenate([ei[0], loops])
    dst_n = np.concatenate([ei[1], loops])

    plan, edata = build_edge_plan(cfg, src_n, dst_n)
    nc = _get_program(cfg, plan)
    if _runner is not None:
        run = _runner
    elif _collect_times is not None:
        run = _run_timed
    else:
        run = _run

    xpad = np.zeros((NCORES * NPC, cfg["IN"]), np.float32)
    xpad[:N] = x
    AA1 = np.concatenate([_block_diag_att(np.asarray(att_src1, np.float32)),
                          _block_diag_att(np.asarray(att_dst1, np.float32))],
                         axis=1)
    AA2 = np.concatenate([_block_diag_att(np.asarray(att_src2, np.float32)),
                          _block_diag_att(np.asarray(att_dst2, np.float32))],
                         axis=1)
    b1bc = np.tile(np.asarray(b1, np.float32)[None, :], (P, 1))
    b2bc = np.tile(np.asarray(b2, np.float32)[None, :], (P, 1))
    W2bf = np.asarray(W2, np.float32).astype(BF16)
    AA2bf = np.concatenate(
        [AA2[k * P:(k + 1) * P] for k in range(D2 // P)],
        axis=1).astype(BF16)

    in_maps = [
        dict(xs=xpad[c * NPC:(c + 1) * NPC],
             W1=np.asarray(W1, np.float32), AA1=AA1, b1bc=b1bc,
             W2=W2bf, AA2=AA2bf, b2bc=b2bc, iota=IOTA,
             gidx=edata["gidx"][c], dstloc=edata["dstloc"][c])
        for c in range(NCORES)
    ]
    res = run(nc, in_maps)
    if _collect_times is not None:
        _collect_times.append(("FUSED", res.exec_time_ns))
    out = np.concatenate([res.results[c]["out2"] for c in range(NCORES)],
                         axis=0)[:N]
    return np.asarray(out, np.float32)


# revision 16
# speedup vs baseline: 3.1070x; 1.0966x over previous
"""GAT (2-layer, PyG-style) Trainium2 Bass kernel, 8-core SPMD, fused.

Strategy (edge parallelism by destination):
  - Add self loops, sort edges by dst, partition dst-node blocks of 128
    across 8 cores (contiguous block ranges).
  - ONE launch. Per layer, each core computes its own slice of the
    node-feature table T (row-per-node: [h bf16 | a_src f32], 256B-multiple
    row stride), an on-device AllGather replicates T to every core, then
    the edge phase gathers T[src] rows per dst block.
  - Edge phase per core: for each of its dst blocks, batched dma_gather
    of T[src] rows (int16 idx limit 32767 -> two gathers split by
    src < 32768), then per 128-edge tile:
      S[e,d] = (dstloc_e == d)                 (DVE tensor_scalar is_equal)
      ST = S^T                                 (PE transpose)
      u = ST.T @ a_dst_block + a_src_gathered  (PE matmul + DVE add)
      p = exp(leaky_relu(u))                   (ACT Lrelu, ACT Exp)
      M = h_gathered * p (per-head broadcast)  (DVE)
      acc[d, :] += S.T @ [M | p]               (PE matmul, PSUM accumulate)
    Segment softmax without max-subtraction (logits are O(10), exact in
    f32: softmax is shift-invariant so this matches the reference).
  - Block epilogue: out = acc[:, :HC] / acc[:, HC:] (per head), + bias,
    ELU (layer 1) or head-mean (layer 2).
  - a_dst values for a core's own dst blocks never travel through the
    table: phase-0 writes them into a persistent SBUF tile directly.

Program layout (single Bass program):
  A0:  x_slice @ W1 -> t1 slice rows [h1|as1] + ad1 SBUF     (distributed)
  CC1: AllGather t1 slice -> full T1 (DRAM)
  E1:  layer-1 edge phase -> h1' slice (DRAM);
       h1' @ W2 -> t2 slice rows [h2|as2] + ad2 SBUF
  CC2: AllGather t2 slice -> full T2 (DRAM)
  E2:  layer-2 edge phase -> out slice (ExternalOutput)
"""

import sys

sys.path.insert(0, "/opt/trn_rl_repo")

import math
import numpy as np
import ml_dtypes

import concourse.bass as bass
import concourse.bacc as bacc
import concourse.tile as tile
from concourse import mybir
from concourse.bass_utils import run_bass_kernel_spmd
from concourse.masks import make_identity

BF16 = ml_dtypes.bfloat16
F32 = mybir.dt.float32
BF = mybir.dt.bfloat16
I16 = mybir.dt.int16
I32 = mybir.dt.int32

P = 128
NCORES = 8
SPLIT = 32768
NEG_SLOPE = 0.2
PAD_DST = 1000.0  # dstloc sentinel: matches no d in [0,128)
IOTA = np.tile(np.arange(P, dtype=np.float32), (P, 1)).astype(ml_dtypes.bfloat16)


def _cfg(N, E, IN, H1, C1, H2, C2):
    nblk = math.ceil(N / P)
    slots = math.ceil(nblk / NCORES)
    return dict(
        N=N, E=E, IN=IN, H1=H1, C1=C1, H2=H2, C2=C2,
        D1=H1 * C1, D2=H2 * C2,
        NBLK=nblk, SLOTS=slots, NPC=slots * P, NPAD=nblk * P,
        # table row lengths in bf16 elems (256B-multiple strides)
        ROW1=_row_elems(H1 * C1 + 2 * H1),  # h bf16 + as f32
        ROW2=_row_elems(H2 * C2 + 2 * H2),
    )


def _row_elems(used_bf16_elems):
    # round row up to a multiple of 128 bf16 elems (256 bytes)
    return ((used_bf16_elems + 127) // 128) * 128


CFG = _cfg(N=50000, E=800000, IN=128, H1=4, C1=32, H2=8, C2=32)


# ---------------------------------------------------------------------------
# Host-side edge plan
# ---------------------------------------------------------------------------

def build_edge_plan(cfg, src, dst):
    """Sort by dst, bucket into (core, slot) dst blocks, split each block's
    edges by src < SPLIT, pad each group to a multiple of 128.

    Returns a static `plan` (identical across cores: per-slot tile counts
    and call descriptors) plus per-core data buffers (gather indices,
    local-dst per tile)."""
    slots, nblk = cfg["SLOTS"], cfg["NBLK"]
    order = np.argsort(dst, kind="stable")
    ss = src[order].astype(np.int64)
    dd = dst[order].astype(np.int64)
    blk_edges = {}
    bounds = np.searchsorted(dd, np.arange(nblk + 1) * P)
    for b in range(nblk):
        lo, hi = bounds[b], bounds[b + 1]
        s_b, d_b = ss[lo:hi], dd[lo:hi]
        a_mask = s_b < SPLIT
        blk_edges[b] = (
            (s_b[a_mask], d_b[a_mask] - b * P),
            (s_b[~a_mask] - SPLIT, d_b[~a_mask] - b * P),
        )

    # static per-slot tile counts (max over cores)
    TA, TB = [], []
    for s in range(slots):
        mxa = mxb = 0
        for c in range(NCORES):
            b = c * slots + s
            if b < nblk:
                mxa = max(mxa, len(blk_edges[b][0][0]))
                mxb = max(mxb, len(blk_edges[b][1][0]))
        ta = max(1, math.ceil(mxa / P))  # >=1 so PSUM is always written
        tb = math.ceil(mxb / P)
        TA.append(ta)
        TB.append(tb)

    # call descriptors: (slot, group, tile_offset_in_slot, ntiles, call_idx)
    # HW cap: a single dma_gather crashes beyond 1024 indices -> <=8 tiles
    MAX_NT = 8
    calls = []
    ttot = 0
    tile_off = []  # per slot, global tile offset
    for s in range(slots):
        tile_off.append(ttot)
        for grp, t0, T in ((0, 0, TA[s]), (1, TA[s], TB[s])):
            off = 0
            while off < T:
                nt = min(MAX_NT, T - off)
                calls.append((s, grp, t0 + off, nt, len(calls)))
                off += nt
        ttot += TA[s] + TB[s]
    ncalls = len(calls)

    # per-core buffers (laid out per (slot, group); gather-call chunking
    # slices this layout at tile boundaries, which lines up exactly)
    Lg = ttot * (P // 16)
    gidx = np.full((NCORES, 16, Lg), -1, np.int16)
    dstloc = np.full((NCORES, P, ttot), PAD_DST, np.float32)
    for c in range(NCORES):
        for s in range(slots):
            b = c * slots + s
            for grp, t0, T in ((0, 0, TA[s]), (1, TA[s], TB[s])):
                if T == 0:
                    continue
                idx_arr = np.zeros(T * P, np.int64)  # pad rows gather row 0
                if b < nblk:
                    sg, dg = blk_edges[b][grp]
                else:
                    sg = dg = np.zeros(0, np.int64)
                n = len(sg)
                assert n <= T * P
                if n:
                    idx_arr[:n] = sg
                    g0 = tile_off[s] + t0
                    pos = np.arange(n)
                    dstloc[c, pos % P, g0 + pos // P] = dg
                col0 = (tile_off[s] + t0) * (P // 16)
                gidx[c, :, col0:col0 + T * (P // 16)] = (
                    idx_arr.reshape(T * (P // 16), 16).T.astype(np.int16)
                )

    slot_tiles = [(TA[s], TB[s]) for s in range(slots)]
    plan = dict(calls=calls, slot_tiles=slot_tiles, tile_off=tile_off,
                ttot=ttot, ncalls=ncalls, Lg=Lg)
    data = dict(
        gidx=np.tile(gidx, (1, 8, 1)),          # [NC, 128, Lg]
        dstloc=dstloc,                           # [NC, 128, ttot] f32
    )
    return plan, data


# ---------------------------------------------------------------------------
# Bass program builder (single fused program)
# ---------------------------------------------------------------------------

def _edge_phase(nc, tc, cfg, plan, layer, T_dram, ado, identb, iota,
                bbc_d, out_dram, gidx_d, dstloc_d):
    """Shared edge phase. layer=1: ELU epilogue -> out_dram [NPC, D1] bf16.
    layer=2: head-mean epilogue -> out_dram [NPC, C2] f32.
    `ado` is a persistent SBUF tile [P, slots*H] with a_dst of own nodes."""
    H = cfg["H1"] if layer == 1 else cfg["H2"]
    HC = cfg["D1"] if layer == 1 else cfg["D2"]
    ROW = cfg["ROW1"] if layer == 1 else cfg["ROW2"]
    NTAB = NCORES * cfg["NPC"]
    slots = cfg["SLOTS"]
    ttot, Lg = plan["ttot"], plan["Lg"]
    Tmax = max(a + b for a, b in plan["slot_tiles"])

    with tc.tile_pool(name=f"ec{layer}", bufs=1) as cp, \
         tc.tile_pool(name=f"gb{layer}", bufs=2) as gp, \
         tc.tile_pool(name=f"ew{layer}", bufs=3) as wp, \
         tc.tile_pool(name=f"es{layer}", bufs=3) as sp, \
         tc.tile_pool(name=f"eps{layer}", bufs=2, space="PSUM") as pp, \
         tc.tile_pool(name=f"eacc{layer}", bufs=2, space="PSUM") as ap:
        gidx = cp.tile([P, Lg], I16, tag="gidx")
        nc.sync.dma_start(out=gidx[:], in_=gidx_d[:])
        dsl = cp.tile([P, ttot], F32, tag="dsl")
        nc.sync.dma_start(out=dsl[:], in_=dstloc_d[:])
        bbc = cp.tile([P, bbc_d.shape[1]], F32, tag="bbc")
        nc.sync.dma_start(out=bbc[:], in_=bbc_d[:])

        calls_by_slot = {}
        for (s, grp, toff, nt, ci) in plan["calls"]:
            calls_by_slot.setdefault(s, []).append((grp, toff, nt, ci))

        for s in range(slots):
            ta, tb = plan["slot_tiles"][s]
            T_s = ta + tb
            g0 = plan["tile_off"][s]
            gb = gp.tile([P, Tmax, ROW], BF, tag="gb")
            for (grp, toff, nt, ci) in calls_by_slot[s]:
                src_tab = T_dram[0:min(SPLIT, NTAB), :] if grp == 0 \
                    else T_dram[SPLIT:NTAB, :]
                nc.gpsimd.dma_gather(
                    out_ap=gb[:, toff:toff + nt, :],
                    in_ap=src_tab,
                    idxs_ap=gidx[:, (g0 + toff) * 8:(g0 + toff + nt) * 8],
                    num_idxs=nt * P,
                    num_idxs_reg=nt * P,
                    elem_size=ROW,
                )
            adb = sp.tile([P, H], BF, tag="adb")
            nc.vector.tensor_copy(out=adb[:], in_=ado[:, s * H:(s + 1) * H])
            acc = ap.tile([P, HC + H], F32, tag="acc")
            for t in range(T_s):
                S = sp.tile([P, P], BF, tag="S")
                nc.vector.tensor_scalar(
                    out=S[:], in0=iota[:], scalar1=dsl[:, g0 + t:g0 + t + 1],
                    scalar2=None, op0=mybir.AluOpType.is_equal)
                STp = pp.tile([P, P], BF, tag="STp")
                nc.tensor.transpose(out=STp[:], in_=S[:], identity=identb[:])
                ST = sp.tile([P, P], BF, tag="ST")
                nc.scalar.copy(out=ST[:], in_=STp[:])
                uE = pp.tile([P, H], F32, tag="uE")
                nc.tensor.matmul(out=uE[:], lhsT=ST[:], rhs=adb[:],
                                 start=True, stop=True)
                u = sp.tile([P, H], F32, tag="u")
                nc.vector.tensor_tensor(
                    out=u[:], in0=uE[:],
                    in1=gb[:, t, HC:HC + 2 * H].bitcast(F32),
                    op=mybir.AluOpType.add)
                lr = sp.tile([P, H], F32, tag="lr")
                nc.vector.scalar_tensor_tensor(
                    out=lr[:], in0=u[:], scalar=NEG_SLOPE, in1=u[:],
                    op0=mybir.AluOpType.mult, op1=mybir.AluOpType.max)
                Mp = sp.tile([P, HC + H], BF, tag="Mp")
                nc.scalar.activation(out=Mp[:, HC:HC + H], in_=lr[:],
                                     func=mybir.ActivationFunctionType.Exp)
                nc.vector.tensor_tensor(
                    out=Mp[:, 0:HC].rearrange("p (h c) -> p h c", h=H),
                    in0=gb[:, t, 0:HC].rearrange("p (h c) -> p h c", h=H),
                    in1=Mp[:, HC:HC + H].to_broadcast([P, H, HC // H]),
                    op=mybir.AluOpType.mult)
                nc.tensor.matmul(out=acc[:], lhsT=S[:], rhs=Mp[:],
                                 start=(t == 0), stop=(t == T_s - 1))
            # epilogue
            rows = slice(s * P, (s + 1) * P)
            rs = wp.tile([P, H], F32, tag="rs")
            nc.vector.reciprocal(out=rs[:], in_=acc[:, HC:HC + H])
            if layer == 1:
                on = wp.tile([P, HC], F32, tag="on")
                nc.vector.tensor_tensor(
                    out=on[:].rearrange("p (h c) -> p h c", h=H),
                    in0=acc[:, 0:HC].rearrange("p (h c) -> p h c", h=H),
                    in1=rs[:].to_broadcast([P, H, HC // H]),
                    op=mybir.AluOpType.mult)
                ob = wp.tile([P, HC], F32, tag="ob")
                nc.vector.tensor_tensor(out=ob[:], in0=on[:], in1=bbc[:],
                                        op=mybir.AluOpType.add)
                # ELU = relu(x) + exp(min(x,0)) - 1
                tmin = wp.tile([P, HC], F32, tag="tmin")
                nc.vector.tensor_scalar_min(out=tmin[:], in0=ob[:],
                                            scalar1=0.0)
                ex = wp.tile([P, HC], F32, tag="ex")
                nc.scalar.activation(out=ex[:], in_=tmin[:],
                                     func=mybir.ActivationFunctionType.Exp)
                rl = wp.tile([P, HC], F32, tag="rl")
                nc.vector.tensor_scalar_max(out=rl[:], in0=ob[:],
                                            scalar1=0.0)
                stage = wp.tile([P, HC], BF, tag="stage1")
                nc.vector.scalar_tensor_tensor(
                    out=stage[:], in0=ex[:], scalar=-1.0, in1=rl[:],
                    op0=mybir.AluOpType.add, op1=mybir.AluOpType.add)
                nc.sync.dma_start(out=out_dram[rows, :], in_=stage[:])
            else:
                C2 = cfg["C2"]
                rs8 = wp.tile([P, H], F32, tag="rs8")
                nc.vector.tensor_scalar_mul(out=rs8[:], in0=rs[:],
                                            scalar1=1.0 / H)
                on = wp.tile([P, HC], F32, tag="on")
                nc.vector.tensor_tensor(
                    out=on[:].rearrange("p (h c) -> p h c", h=H),
                    in0=acc[:, 0:HC].rearrange("p (h c) -> p h c", h=H),
                    in1=rs8[:].to_broadcast([P, H, C2]),
                    op=mybir.AluOpType.mult)
                red = wp.tile([P, C2], F32, tag="red")
                nc.vector.reduce_sum(
                    out=red[:],
                    in_=on[:].rearrange("p (h c) -> p c h", h=H),
                    axis=mybir.AxisListType.X)
                stage = wp.tile([P, C2], F32, tag="stage2")
                nc.vector.tensor_tensor(out=stage[:], in0=red[:], in1=bbc[:],
                                        op=mybir.AluOpType.add)
                nc.sync.dma_start(out=out_dram[rows, :], in_=stage[:])


def build_fused(cfg, plan):
    NPC, IN = cfg["NPC"], cfg["IN"]
    D1, D2, H1, H2, C2 = cfg["D1"], cfg["D2"], cfg["H1"], cfg["H2"], cfg["C2"]
    ROW1, ROW2, slots = cfg["ROW1"], cfg["ROW2"], cfg["SLOTS"]
    NTAB = NCORES * NPC

    nc = bacc.Bacc("TRN2", target_bir_lowering=False, debug=False,
                   num_devices=NCORES)
    xs = nc.declare_dram_parameter("xs", [NPC, IN], F32, isOutput=False)
    W1 = nc.declare_dram_parameter("W1", [IN, D1], F32, isOutput=False)
    AA1 = nc.declare_dram_parameter("AA1", [D1, 2 * H1], F32, isOutput=False)
    b1 = nc.declare_dram_parameter("b1bc", [P, D1], F32, isOutput=False)
    W2 = nc.declare_dram_parameter("W2", [D1, D2], BF, isOutput=False)
    AA2 = nc.declare_dram_parameter("AA2", [P, (D2 // P) * 2 * H2], BF,
                                    isOutput=False)
    b2 = nc.declare_dram_parameter("b2bc", [P, C2], F32, isOutput=False)
    io = nc.declare_dram_parameter("iota", [P, P], BF, isOutput=False)
    gidx_d = nc.declare_dram_parameter("gidx", [P, plan["Lg"]], I16,
                                       isOutput=False)
    dstloc_d = nc.declare_dram_parameter("dstloc", [P, plan["ttot"]], F32,
                                         isOutput=False)
    out2 = nc.declare_dram_parameter("out2", [NPC, C2], F32, isOutput=True)

    groups = [list(range(NCORES))]

    with tile.TileContext(nc) as tc:
        with tc.tile_pool(name="dram", bufs=1, space="DRAM") as dp, \
             tc.tile_pool(name="pers", bufs=1) as pers:
            t1s_d = dp.tile([NPC, ROW1], BF, tag="t1s")
            T1full = dp.tile([NTAB, ROW1], BF, tag="T1full",
                             addr_space="Shared")
            h1_d = dp.tile([NPC, D1], BF, tag="h1d")
            t2s_d = dp.tile([NPC, ROW2], BF, tag="t2s")
            T2full = dp.tile([NTAB, ROW2], BF, tag="T2full",
                             addr_space="Shared")

            ado1 = pers.tile([P, slots * H1], F32, tag="ado1")
            ado2 = pers.tile([P, slots * H2], F32, tag="ado2")
            identf = pers.tile([P, P], F32, tag="identf")
            make_identity(nc, identf[:])
            identb = pers.tile([P, P], BF, tag="identb")
            make_identity(nc, identb[:])
            iota = pers.tile([P, P], BF, tag="iota")
            nc.sync.dma_start(out=iota[:], in_=io[:])

            # ---- phase A0: own nodes -> t1 slice rows [h1|as1], ad1 SBUF
            with tc.tile_pool(name="a0c", bufs=1) as cp, \
                 tc.tile_pool(name="a0w", bufs=3) as wp, \
                 tc.tile_pool(name="a0p", bufs=1, space="PSUM") as pp:
                w1 = cp.tile([IN, D1], F32, tag="w1")
                nc.sync.dma_start(out=w1[:], in_=W1[:])
                aa1 = cp.tile([D1, 2 * H1], F32, tag="aa1")
                nc.sync.dma_start(out=aa1[:], in_=AA1[:])
                for nt in range(slots):
                    rows = slice(nt * P, (nt + 1) * P)
                    xt = wp.tile([P, IN], F32, tag="xt")
                    nc.sync.dma_start(out=xt[:], in_=xs[rows, :])
                    xTp = pp.tile([P, P], F32, tag="xTp")
                    nc.tensor.transpose(out=xTp[:], in_=xt[:],
                                        identity=identf[:])
                    xT = wp.tile([P, P], F32, tag="xT")
                    nc.vector.tensor_copy(out=xT[:], in_=xTp[:])
                    hTp = pp.tile([P, P], F32, tag="hTp")
                    nc.tensor.matmul(out=hTp[:], lhsT=w1[:], rhs=xT[:],
                                     start=True, stop=True)
                    hT = wp.tile([P, P], F32, tag="hT")
                    nc.vector.tensor_copy(out=hT[:], in_=hTp[:])
                    aaTp = pp.tile([2 * H1, P], F32, tag="aaTp")
                    nc.tensor.matmul(out=aaTp[:], lhsT=aa1[:], rhs=hT[:],
                                     start=True, stop=True)
                    aaT = wp.tile([2 * H1, P], F32, tag="aaT")
                    nc.scalar.copy(out=aaT[:], in_=aaTp[:])
                    hp = pp.tile([P, P], F32, tag="hp")
                    nc.tensor.transpose(out=hp[:], in_=hT[:],
                                        identity=identf[:])
                    aap = pp.tile([P, 2 * H1], F32, tag="aap")
                    nc.tensor.matmul(out=aap[:], lhsT=aaT[:],
                                     rhs=identf[0:2 * H1, 0:2 * H1],
                                     start=True, stop=True)
                    stage = wp.tile([P, ROW1], BF, tag="stage")
                    nc.vector.tensor_copy(out=stage[:, 0:D1], in_=hp[:])
                    nc.scalar.copy(
                        out=stage[:, D1:D1 + 2 * H1].bitcast(F32),
                        in_=aap[:, 0:H1])
                    nc.vector.tensor_copy(
                        out=ado1[:, nt * H1:(nt + 1) * H1],
                        in_=aap[:, H1:2 * H1])
                    nc.sync.dma_start(out=t1s_d[rows, :], in_=stage[:])

            # ---- CC1: AllGather t1 slice -> full T1
            nc.gpsimd.collective_compute(
                "AllGather", mybir.AluOpType.bypass, replica_groups=groups,
                ins=[t1s_d[:].opt()], outs=[T1full[:].opt()])

            # ---- E1: layer-1 edge phase -> h1' slice
            _edge_phase(nc, tc, cfg, plan, 1, T1full, ado1, identb, iota,
                        b1, h1_d, gidx_d, dstloc_d)

            # ---- phase-0 of layer 2 on own h1' slice
            with tc.tile_pool(name="p0c", bufs=1) as cp, \
                 tc.tile_pool(name="p0w", bufs=3) as wp, \
                 tc.tile_pool(name="p0p", bufs=1, space="PSUM") as pp:
                w2 = cp.tile([D1, D2], BF, tag="w2")
                nc.sync.dma_start(out=w2[:], in_=W2[:])
                nchunk = D2 // P
                aa2 = cp.tile([P, nchunk * 2 * H2], BF, tag="aa2")
                nc.sync.dma_start(out=aa2[:], in_=AA2[:])
                for nt in range(slots):
                    rows = slice(nt * P, (nt + 1) * P)
                    h1T = wp.tile([P, P], BF, tag="h1T")
                    nc.sync.dma_start_transpose(out=h1T[:], in_=h1_d[rows, :])
                    h2T = []
                    for k in range(nchunk):
                        h2Tp = pp.tile([P, P], F32, tag=f"h2Tp{k}")
                        nc.tensor.matmul(out=h2Tp[:],
                                         lhsT=w2[:, k * P:(k + 1) * P],
                                         rhs=h1T[:], start=True, stop=True)
                        h2Tk = wp.tile([P, P], BF, tag=f"h2T{k}")
                        nc.vector.tensor_copy(out=h2Tk[:], in_=h2Tp[:])
                        h2T.append(h2Tk)
                    aaTp = pp.tile([2 * H2, P], F32, tag="aaTp2")
                    for k in range(nchunk):
                        nc.tensor.matmul(
                            out=aaTp[:],
                            lhsT=aa2[:, k * 2 * H2:(k + 1) * 2 * H2],
                            rhs=h2T[k][:],
                            start=(k == 0), stop=(k == nchunk - 1))
                    aaT = wp.tile([2 * H2, P], BF, tag="aaT2")
                    nc.scalar.copy(out=aaT[:], in_=aaTp[:])
                    aap = pp.tile([P, 2 * H2], F32, tag="aap2")
                    nc.tensor.matmul(out=aap[:], lhsT=aaT[:],
                                     rhs=identb[0:2 * H2, 0:2 * H2],
                                     start=True, stop=True)
                    stage = wp.tile([P, ROW2], BF, tag="stage0b")
                    for k in range(nchunk):
                        hp = pp.tile([P, P], BF, tag=f"hp2{k}")
                        nc.tensor.transpose(out=hp[:], in_=h2T[k][:],
                                            identity=identb[:])
                        nc.vector.tensor_copy(out=stage[:, k * P:(k + 1) * P],
                                              in_=hp[:])
                    nc.scalar.copy(
                        out=stage[:, D2:D2 + 2 * H2].bitcast(F32),
                        in_=aap[:, 0:H2])
                    nc.vector.tensor_copy(
                        out=ado2[:, nt * H2:(nt + 1) * H2],
                        in_=aap[:, H2:2 * H2])
                    nc.sync.dma_start(out=t2s_d[rows, :], in_=stage[:])

            # ---- CC2: AllGather t2 slice -> full T2
            nc.gpsimd.collective_compute(
                "AllGather", mybir.AluOpType.bypass, replica_groups=groups,
                ins=[t2s_d[:].opt()], outs=[T2full[:].opt()])

            # ---- E2: layer-2 edge phase -> out slice
            _edge_phase(nc, tc, cfg, plan, 2, T2full, ado2, identb, iota,
                        b2, out2, gidx_d, dstloc_d)
    nc.compile()
    return nc


# ---------------------------------------------------------------------------
# Host orchestration
# ---------------------------------------------------------------------------

def _block_diag_att(att):
    """att [H, C] -> [H*C, H] block diagonal."""
    H, C = att.shape
    out = np.zeros((H * C, H), np.float32)
    for h in range(H):
        out[h * C:(h + 1) * C, h] = att[h]
    return out


_CACHE = {}


def _get_program(cfg, plan):
    key = (cfg["N"], cfg["E"], tuple(plan["slot_tiles"]), plan["ncalls"])
    if key not in _CACHE:
        _CACHE[key] = build_fused(cfg, plan)
    return _CACHE[key]


def _run(nc, in_maps, **kw):
    res = run_bass_kernel_spmd(nc, in_maps, list(range(NCORES)), **kw)
    return res


def _run_timed(nc, in_maps, n_iters=3):
    """Like bass2jax.run_bass_via_pjrt but with device-resident inputs and
    repeated timed executes (min wall over n_iters after warmup)."""
    import time
    import jax
    from jax.sharding import Mesh, PartitionSpec, NamedSharding
    from jax.experimental.shard_map import shard_map
    from concourse.bass2jax import _bass_exec_p, partition_id_tensor, \
        install_neuronx_cc_hook

    install_neuronx_cc_hook()
    n_cores = len(in_maps)
    partition_name = nc.partition_id_tensor.name if nc.partition_id_tensor \
        else None
    in_names, out_names, out_avals, zero_outs = [], [], [], []
    for alloc in nc.m.functions[0].allocations:
        if not isinstance(alloc, mybir.MemoryLocationSet):
            continue
        name = alloc.memorylocations[0].name
        if alloc.kind == "ExternalInput":
            if name != partition_name:
                in_names.append(name)
        elif alloc.kind == "ExternalOutput":
            shape = tuple(alloc.tensor_shape)
            dtype = mybir.dt.np(alloc.dtype)
            out_names.append(name)
            out_avals.append(jax.core.ShapedArray(shape, dtype))
            zero_outs.append(np.zeros(shape, dtype))
    n_params = len(in_names)
    n_outs = len(out_avals)
    in_names_all = in_names + out_names
    if partition_name is not None:
        in_names_all = in_names_all + [partition_name]

    def _body(*args):
        operands = list(args)
        if partition_name is not None:
            operands.append(partition_id_tensor())
        return tuple(_bass_exec_p.bind(
            *operands, out_avals=tuple(out_avals),
            in_names=tuple(in_names_all), out_names=tuple(out_names),
            lowering_input_output_aliases=(),
            sim_require_finite=True, sim_require_nnan=True, nc=nc))

    devices = jax.devices()[:n_cores]
    mesh = Mesh(np.asarray(devices), ("core",))
    spec = PartitionSpec("core")
    # Donate the zero output buffers: NEFFs with collectives depend on the
    # donation mechanism (outputs must alias the pre-zeroed operands).
    donate = tuple(range(n_params, n_params + n_outs))
    sharded = jax.jit(
        shard_map(_body, mesh=mesh, in_specs=(spec,) * (n_params + n_outs),
                  out_specs=(spec,) * n_outs, check_rep=False),
        donate_argnums=donate, keep_unused=True)
    sh = NamedSharding(mesh, spec)
    dev_in = [
        jax.device_put(
            np.concatenate([np.asarray(in_maps[c][nm]) for c in
                            range(n_cores)], axis=0), sh)
        for nm in in_names
    ]
    host_zeros = [
        np.zeros((n_cores * z.shape[0], *z.shape[1:]), z.dtype)
        for z in zero_outs
    ]

    def _fresh_zeros():
        dz = [jax.device_put(z, sh) for z in host_zeros]
        jax.block_until_ready(dz)
        return dz

    out = sharded(*dev_in, *_fresh_zeros())  # warmup + compile
    jax.block_until_ready(out)
    wall = []
    for _ in range(n_iters):
        dz = _fresh_zeros()
        t0 = time.perf_counter()
        o = sharded(*dev_in, *dz)
        jax.block_until_ready(o)
        wall.append(time.perf_counter() - t0)
    results = [
        {nm: np.asarray(out[i]).reshape(n_cores, *out_avals[i].shape)[c]
         for i, nm in enumerate(out_names)}
        for c in range(n_cores)
    ]

    class R:
        pass
    r = R()
    r.results = results
    r.exec_time_ns = int(min(wall) * 1e9)
    r.wall_all = wall
    return r


def kernel(x, edge_index, W1, att_src1, att_dst1, b1, W2, att_src2,
           att_dst2, b2, _collect_times=None, _cfg_override=None,
           _runner=None):
    cfg = _cfg_override or CFG
    N, NPC = cfg["N"], cfg["NPC"]
    D2, H2 = cfg["D2"], cfg["H2"]

    x = np.asarray(x, np.float32)
    ei = np.asarray(edge_index)
    loops = np.arange(N, dtype=ei.dtype)
    src_n = np.concatenate([ei[0], loops])
    dst_n = np.concatenate([ei[1], loops])

    plan, edata = build_edge_plan(cfg, src_n, dst_n)
    nc = _get_program(cfg, plan)
    if _runner is not None:
        run = _runner
    elif _collect_times is not None:
        run = _run_timed
    else:
        run = _run

    xpad = np.zeros((NCORES * NPC, cfg["IN"]), np.float32)
    xpad[:N] = x
    AA1 = np.concatenate([_block_diag_att(np.asarray(att_src1, np.float32)),
                          _block_diag_att(np.asarray(att_dst1, np.float32))],
                         axis=1)
    AA2 = np.concatenate([_block_diag_att(np.asarray(att_src2, np.float32)),
                          _block_diag_att(np.asarray(att_dst2, np.float32))],
                         axis=1)
    b1bc = np.tile(np.asarray(b1, np.float32)[None, :], (P, 1))
    b2bc = np.tile(np.asarray(b2, np.float32)[None, :], (P, 1))
    W2bf = np.asarray(W2, np.float32).astype(BF16)
    AA2bf = np.concatenate(
        [AA2[k * P:(k + 1) * P] for k in range(D2 // P)],
        axis=1).astype(BF16)

    in_maps = [
        dict(xs=xpad[c * NPC:(c + 1) * NPC],
             W1=np.asarray(W1, np.float32), AA1=AA1, b1bc=b1bc,
             W2=W2bf, AA2=AA2bf, b2bc=b2bc, iota=IOTA,
             gidx=edata["gidx"][c], dstloc=edata["dstloc"][c])
        for c in range(NCORES)
    ]
    res = run(nc, in_maps)
    if _collect_times is not None:
        _collect_times.append(("FUSED", res.exec_time_ns))
    out = np.concatenate([res.results[c]["out2"] for c in range(NCORES)],
                         axis=0)[:N]
    return np.asarray(out, np.float32)
